# revision 17
# baseline (speedup 1.0000x reference)
"""Bahdanau attention kernel for Trainium2 (8 NeuronCores, SPMD data-parallel).

Reference computation (per batch b):
    f_proj = features[b] @ W1_w + W1_b            # [T, U]
    h_proj = hidden[b] @ W2_w + W2_b              # [U]
    score  = tanh(f_proj + h_proj) @ V_w + V_b    # [T]
    attn   = softmax(score)                       # [T]
    context[b] = sum_t attn[t] * features[b, t]   # [D]

Sharding: data-parallel over batch (64 batches / 8 cores = 8 per core),
weights replicated.

Per-core dataflow:
  - F tiles [128(t), 512(d)] are DMA'd in natively, PE-transposed
    (via identity matmul, float32r = 1.5 cyc/row) into F^T [128(d), t].
  - main matmul (float32r, full PE rate) computes f_proj TRANSPOSED:
    [u(part), t(free)] = W1_chunk^T @ F^T, so the (W1_b + h_proj) bias
    is a per-partition scalar fused into the ACT Tanh instruction.
  - Tanh output is bf16 (costs ~6e-4 relative error, enables the next
    step); per batch, score^T rows are computed with 4-way col-tiled
    concurrent bf16 matmuls (fp32r forbids nonzero PSUM base
    partitions, bf16 doesn't).
  - ACT Exp with fused accum_out produces e = exp(score + V_b) and its
    row sums. No max-subtraction: |score| <= ||V||_1 + |V_b| ~ 18,
    safely inside fp32 exp range.
  - e rows transpose to columns with tiny [1,0]-identity-row matmuls
    (pairs keep fp32r ISA patterns even); context accumulates as
    e_col^T @ F_native (fp32r); final scale by 1/sum(e).
  - Score/context stages are deferred to the start of the next batch so
    their ACT/DVE round-trips overlap the dense transpose/main work.
"""

import sys

for _p in ("/opt/trn_rl_repo", "/opt/pypackages"):
    if _p not in sys.path:
        sys.path.insert(0, _p)

import numpy as np

B, T, D, U = 64, 2048, 512, 512
NCORES = 8
BPC = B // NCORES          # batches per core
PART = 128
DC = D // PART             # 4 contraction chunks
UC = U // PART             # 4 u chunks
TCHUNK = 512               # t columns per main-matmul group
TILES_PER_CHUNK = TCHUNK // PART          # 4
NCHUNKS = (BPC * T) // TCHUNK             # 32
CHUNKS_PER_BATCH = T // TCHUNK            # 4
TPB = T // PART                           # 16 t-tiles per batch

MM_DT_NAME = "float32r"    # dtype tag for the fp32-path matmuls


_BUILD_CACHE = {}


def build_bass(mm_dt_name=MM_DT_NAME):
    """Build + compile the per-core Bass program (same on all cores)."""
    if mm_dt_name in _BUILD_CACHE:
        return _BUILD_CACHE[mm_dt_name]

    import concourse.mybir as mybir
    import concourse.tile as tile
    from concourse import bacc
    from concourse.bass import ts
    from concourse.masks import make_identity

    f32 = mybir.dt.float32
    bf16 = mybir.dt.bfloat16
    mdt = getattr(mybir.dt, mm_dt_name)
    ACT = mybir.ActivationFunctionType
    AX = mybir.AxisListType

    nc = bacc.Bacc("TRN2", target_bir_lowering=False, debug=False)

    feat = nc.dram_tensor("features", [BPC, T, D], mdt, kind="ExternalInput")
    hid = nc.dram_tensor("hidden", [BPC, D], mdt, kind="ExternalInput")
    w1 = nc.dram_tensor("W1_w", [D, U], mdt, kind="ExternalInput")
    b1 = nc.dram_tensor("W1_b", [U], f32, kind="ExternalInput")
    w2 = nc.dram_tensor("W2_w", [D, U], mdt, kind="ExternalInput")
    b2 = nc.dram_tensor("W2_b", [U], f32, kind="ExternalInput")
    vw = nc.dram_tensor("V_w", [U, 1], mdt, kind="ExternalInput")
    vb = nc.dram_tensor("V_b", [1], f32, kind="ExternalInput")
    out = nc.dram_tensor("context", [BPC, D], f32, kind="ExternalOutput")

    with tile.TileContext(nc) as tc:
        with (
            tc.tile_pool(name="consts", bufs=1) as consts,
            tc.tile_pool(name="fpool", bufs=26) as fpool,
            tc.tile_pool(name="ftb", bufs=3) as ftb,
            tc.tile_pool(name="tanh", bufs=6) as tanhp,
            tc.tile_pool(name="small", bufs=3) as small,
            tc.tile_pool(name="outp", bufs=2) as outp,
            tc.tile_pool(name="ps_mm", bufs=3, space="PSUM") as ps_mm,
            tc.tile_pool(name="ps_t", bufs=3, space="PSUM") as ps_t,
            tc.tile_pool(name="ps_s", bufs=1, space="PSUM") as ps_s,
            tc.tile_pool(name="ps_c", bufs=1, space="PSUM") as ps_c,
        ):
            # ---------------- constants / setup ----------------
            ident_f32 = consts.tile([PART, PART], f32)
            make_identity(nc, ident_f32)
            ident = consts.tile([PART, PART], mdt)
            nc.vector.tensor_copy(ident, ident_f32)

            # preload the first two chunks' F tiles so the PE can start on
            # their transposes before the (large) weight DMAs complete
            preloaded = {}
            for pch in (0, 1):
                pb = pch // CHUNKS_PER_BATCH
                pt0 = (pch % CHUNKS_PER_BATCH) * TCHUNK
                tiles = []
                for j in range(TILES_PER_CHUNK):
                    f_pre = fpool.tile([PART, D], mdt, tag="F", name=f"f_pre_{pch}_{j}")
                    nc.sync.dma_start(
                        out=f_pre,
                        in_=feat.ap()[pb, pt0 + j * PART : pt0 + (j + 1) * PART, :],
                    )
                    tiles.append(f_pre)
                preloaded[pch] = tiles

            w1_sb = consts.tile([PART, DC, U], mdt)
            nc.sync.dma_start(out=w1_sb, in_=w1.ap().rearrange("(c p) u -> p c u", p=PART))
            w2_sb = consts.tile([PART, DC, U], mdt)
            nc.sync.dma_start(out=w2_sb, in_=w2.ap().rearrange("(c p) u -> p c u", p=PART))
            v_sb = consts.tile([PART, UC], mdt)
            nc.sync.dma_start(out=v_sb, in_=vw.ap().rearrange("(c p) one -> p (c one)", p=PART))
            v_bf = consts.tile([PART, UC], bf16)
            nc.vector.tensor_copy(v_bf, v_sb)
            vb_sb = consts.tile([1, 1], f32)
            nc.sync.dma_start(out=vb_sb, in_=vb.ap().rearrange("(one x) -> one x", one=1))
            import concourse.bass as bass_mod
            vb4_sb = consts.tile([97, 1], f32)
            nc.gpsimd.dma_start(
                out=vb4_sb,
                in_=bass_mod.AP(tensor=vb.ap().tensor, offset=0, ap=[[0, 97], [1, 1]]),
            )
            ones97 = consts.tile([97, 1], f32)
            nc.vector.memset(ones97, 1.0)

            # W1_b + W2_b as per-partition columns [128, uc]
            b1_sb = consts.tile([PART, UC], f32)
            nc.sync.dma_start(out=b1_sb, in_=b1.ap().rearrange("(c p) -> p c", p=PART))
            b2_sb = consts.tile([PART, UC], f32)
            nc.sync.dma_start(out=b2_sb, in_=b2.ap().rearrange("(c p) -> p c", p=PART))
            b12_sb = consts.tile([PART, UC], f32)
            nc.vector.tensor_add(b12_sb, b1_sb, b2_sb)

            # hidden [BPC, D] -> hiddenT [128(d), dc, BPC]
            hid_sb = consts.tile([BPC, D], mdt)
            nc.sync.dma_start(out=hid_sb, in_=hid.ap())
            hidT_sb = consts.tile([PART, DC, BPC], mdt)
            bias_cols = consts.tile([PART, UC, BPC], f32)

            def emit_setup():
                # emitted after chunk 0's transposes so the PE isn't blocked
                # on the weight/hidden DMAs at kernel start
                for dc in range(DC):
                    ps_h = ps_t.tile([PART, TCHUNK], mdt, tag="T", name="ps_h")
                    nc.tensor.transpose(
                        ps_h[:, 0:BPC], hid_sb[:, ts(dc, PART)], ident[0:BPC, 0:BPC]
                    )
                    nc.vector.tensor_copy(hidT_sb[:, dc, :], ps_h[:, 0:BPC])
                # h_projT[u, b] = sum_dc W2[dc]^T @ hiddenT[dc]  (+W2_b+W1_b)
                for uc in range(UC):
                    ps_h = ps_t.tile([PART, TCHUNK], f32, tag="T", name="ps_h2")
                    for dc in range(DC):
                        nc.tensor.matmul(
                            ps_h[:, 0:BPC],
                            w2_sb[:, dc, ts(uc, PART)],
                            hidT_sb[:, dc, :],
                            start=(dc == 0),
                            stop=(dc == DC - 1),
                        )
                    nc.vector.tensor_scalar_add(
                        bias_cols[:, uc, :], ps_h[:, 0:BPC], b12_sb[:, uc : uc + 1]
                    )

            # -------- batch-level epilogue (score / softmax / context) --------
            ep = {}

            def emit_scores_batch(eb):
                # 3-way col-tiled concurrent bf16 V-dot (col groups 0/32/64;
                # group 96 avoided - array quadrant 3 is buggy) plus a fourth
                # serial row in its own PSUM bank. Everything lane-aligned:
                # ACT accumulators/bias live on the same partitions as inputs.
                ps_sc4 = ps_s.tile([97, TCHUNK], f32, tag="score", name="ps_sc4")
                ps_sc_x = ps_c.tile([1, TCHUNK], f32, tag="ctx", name="ps_sc_x")
                for uc in range(UC):
                    for cib in range(CHUNKS_PER_BATCH):
                        if cib < 3:
                            out_ap = ps_sc4[32 * cib : 32 * cib + 1, :]
                            tp = (0, 32 * cib)
                        else:
                            out_ap = ps_sc_x
                            tp = (0, 0)
                        nc.tensor.matmul(
                            out_ap,
                            v_bf[:, uc : uc + 1],
                            ep["tanh"][cib][:, uc, :],
                            start=(uc == 0),
                            stop=(uc == UC - 1),
                            tile_position=tp,
                        )
                # e = exp(score + V_b) rows; per-row sums land lane-aligned
                # in s4 (zeroed), combined later via a ones-matmul.
                s4 = small.tile([97, 2], f32, tag="ssum", name="s4")
                nc.vector.memset(s4, 0.0)
                e4_sb = small.tile([97, TCHUNK], mdt, tag="e_sb", name="e4_sb")
                e_x = small.tile([1, TCHUNK], mdt, tag="e_x", name="e_x")
                for cib in range(3):
                    p0 = 32 * cib
                    nc.scalar.activation(
                        e4_sb[p0 : p0 + 1, :],
                        ps_sc4[p0 : p0 + 1, :],
                        ACT.Exp,
                        bias=vb4_sb[p0 : p0 + 1, :],
                        accum_out=s4[p0 : p0 + 1, 0:1],
                    )
                nc.scalar.activation(
                    e_x,
                    ps_sc_x,
                    ACT.Exp,
                    bias=vb_sb,
                    accum_out=s4[0:1, 1:2],
                )
                ep["s4"] = s4
                ep["e4_sb"] = e4_sb
                ep["e_x"] = e_x

            def emit_etr_batch(eb):
                # transpose e rows -> columns via [1,0]-identity-row matmuls
                # (pairs of output columns keep fp32r ISA patterns even)
                e4_sb = ep["e4_sb"]
                e_x = ep["e_x"]
                ps_e = ps_t.tile([PART, 2 * TPB], f32, tag="T", name="ps_e")
                for k in range(TPB):
                    cib, j = divmod(k, TILES_PER_CHUNK)
                    p0 = 32 * cib if cib < 3 else 0
                    row = e4_sb[p0 : p0 + 1, ts(j, PART)] if cib < 3 else e_x[0:1, ts(j, PART)]
                    nc.tensor.matmul(
                        ps_e[:, 2 * k : 2 * k + 2],
                        row,
                        ident[p0 : p0 + 1, p0 : p0 + 2],
                        start=True,
                        stop=True,
                        tile_position=(p0, 0),
                    )
                e_colT = small.tile([PART, TPB], mdt, tag="e_col", name="e_colT")
                nc.vector.tensor_copy(
                    e_colT, ps_e.rearrange("p (k two) -> p two k", two=2)[:, 0, :]
                )
                ep["e_colT"] = e_colT

            def emit_ctx_batch(eb):
                # context accumulation over the batch's 16 tiles + finalize
                e_colT = ep["e_colT"]
                ps_ctx = ps_c.tile([1, D], f32, tag="ctx", name="ps_ctx")
                for k in range(TPB):
                    nc.tensor.matmul(
                        ps_ctx,
                        e_colT[:, k : k + 1],
                        ep["f_tiles"][k],
                        start=(k == 0),
                        stop=(k == TPB - 1),
                    )
                ps_tiny = ps_t.tile([1, 2], f32, tag="T", name="ps_tiny")
                nc.tensor.matmul(
                    ps_tiny, ones97, ep["s4"], start=True, stop=True
                )
                ssum = small.tile([1, 1], f32, tag="ssum1", name="ssum")
                nc.vector.reduce_sum(ssum, ps_tiny, axis=AX.X)
                rec = small.tile([1, 1], f32, tag="rec", name="rec")
                nc.vector.reciprocal(rec, ssum)
                ctx_sb = outp.tile([1, D], f32, tag="ctx_sb", name="ctx_sb")
                nc.vector.tensor_scalar_mul(ctx_sb, ps_ctx, rec)
                nc.sync.dma_start(out=out.ap()[eb : eb + 1, :], in_=ctx_sb)

            # ---------------- main loop ----------------
            cur_tanh = []   # tanh tiles of the in-flight batch
            cur_f = []      # f tiles of the in-flight batch

            for chunk in range(NCHUNKS):
                b = chunk // CHUNKS_PER_BATCH
                cib = chunk % CHUNKS_PER_BATCH
                t0 = cib * TCHUNK

                boundary = cib == 0 and chunk > 0
                if boundary:
                    ep["tanh"] = cur_tanh
                    ep["f_tiles"] = cur_f
                    cur_tanh, cur_f = [], []
                    emit_scores_batch(b - 1)

                # S1: load + transpose 4 F tiles
                f_tiles = []
                ftile_big = ftb.tile([PART, DC, TCHUNK], mdt, tag="FT")
                for j in range(TILES_PER_CHUNK):
                    if chunk in preloaded:
                        f_ij = preloaded[chunk][j]
                    else:
                        f_ij = fpool.tile([PART, D], mdt, tag="F", name="f_ij")
                        nc.sync.dma_start(
                            out=f_ij,
                            in_=feat.ap()[b, t0 + j * PART : t0 + (j + 1) * PART, :],
                        )
                    f_tiles.append(f_ij)
                    ps_tr = ps_t.tile([PART, TCHUNK], mdt, tag="T")
                    for dc in range(DC):
                        nc.tensor.transpose(
                            ps_tr[:, ts(dc, PART)], f_ij[:, ts(dc, PART)], ident
                        )
                    nc.vector.tensor_copy(
                        ftile_big[:, :, ts(j, PART)],
                        ps_tr.rearrange("p (c t) -> p c t", c=DC),
                    )
                if chunk == 0:
                    emit_setup()
                if boundary:
                    emit_etr_batch(b - 1)

                # S2: main matmul + tanh (transposed layout [u, t], bf16 out)
                tanh_sb = tanhp.tile([PART, UC, TCHUNK], bf16, tag="tanh")
                for uc in range(UC):
                    ps_f = ps_mm.tile([PART, TCHUNK], f32, tag="mm")
                    for dc in range(DC):
                        nc.tensor.matmul(
                            ps_f,
                            w1_sb[:, dc, ts(uc, PART)],
                            ftile_big[:, dc, :],
                            start=(dc == 0),
                            stop=(dc == DC - 1),
                        )
                    nc.scalar.activation(
                        tanh_sb[:, uc, :],
                        ps_f,
                        ACT.Tanh,
                        bias=bias_cols[:, uc, b : b + 1],
                    )
                cur_tanh.append(tanh_sb)
                cur_f.extend(f_tiles)

                if boundary:
                    emit_ctx_batch(b - 1)

            # flush the last batch
            ep["tanh"] = cur_tanh
            ep["f_tiles"] = cur_f
            emit_scores_batch(BPC - 1)
            emit_etr_batch(BPC - 1)
            emit_ctx_batch(BPC - 1)

    nc.compile()
    _BUILD_CACHE[mm_dt_name] = nc
    return nc


def kernel(**inputs):
    from concourse.bass_utils import run_bass_kernel_spmd

    nc = build_bass()

    feat = np.ascontiguousarray(np.asarray(inputs["features"], dtype=np.float32))
    hid = np.ascontiguousarray(np.asarray(inputs["hidden"], dtype=np.float32))
    shared = {
        k: np.ascontiguousarray(np.asarray(inputs[k], dtype=np.float32))
        for k in ("W1_w", "W1_b", "W2_w", "W2_b", "V_w", "V_b")
    }
    in_maps = []
    for c in range(NCORES):
        m = dict(shared)
        m["features"] = feat[c * BPC : (c + 1) * BPC]
        m["hidden"] = hid[c * BPC : (c + 1) * BPC]
        in_maps.append(m)

    res = run_bass_kernel_spmd(nc, in_maps, list(range(NCORES)))
    return np.concatenate([res.results[c]["context"] for c in range(NCORES)], axis=0)
